# revision 1
# baseline (speedup 1.0000x reference)
"""Bass/Tile TRN2 kernel for nn_CRMF_35296041239144.

Social-LSTM-style decoder: mapping MLP on K x B hidden states, then a
12-step LSTM recurrence (hard-sigmoid gates, clipped cell) with a 2-dim
output projection per step.

Sharding: batch 2048 -> 8 cores x 256. Per core rows = K*Bc = 5120.
State is kept transposed [H=128 partitions, rows free] so the recurrent
matmul needs no transposes: gates[gate_unit, row] = W_hh_block.T @ h.
The x-term + bias + hard-sigmoid affine offset are folded into the same
PSUM accumulation via contraction-3 matmuls (x0, x1, 1) placed in
distinct PE row-groups so all four run concurrently.

hard_sigmoid(z) = clip(z/6 + 0.5, 0, 1): the 1/6 scale and +0.5 offset
are pre-folded into the i/f/o weight blocks host-side, so on device the
gate nonlinearity is a single dual-op (max 0, min 1) tensor_scalar.
"""

import numpy as np

import concourse.bass as bass
import concourse.bacc as bacc
import concourse.tile as tile
from concourse import mybir
from concourse.bass_utils import run_bass_kernel_spmd

OBS_LEN, K, B, H, MID, NC_OUT, CIN = 12, 20, 2048, 128, 256, 2, 3
NCORES = 8
BC = B // NCORES            # 256 batch rows per core
ROWS = K * BC               # 5120 rows per core (k-major: r = k*BC + b)
CHUNK = 512
NCH = ROWS // CHUNK         # 10
NTILE = ROWS // 128         # 40 transpose tiles

F32 = mybir.dt.float32
BF16 = mybir.dt.bfloat16
AF = mybir.ActivationFunctionType
OP = mybir.AluOpType

# gate order used on device: [i, f, o, g]; source block order in w_ih/w_hh
# is [i, f, g, o] (reference splits gates into i,f,g,o).
SRC_BLOCK = [0, 1, 3, 2]


def build_nc(reps: int = 1):
    nc = bacc.Bacc("TRN2", target_bir_lowering=False, debug=False)

    ph = nc.dram_tensor("ph", [ROWS, H], F32, kind="ExternalInput")
    xr = nc.dram_tensor("xr", [3, OBS_LEN, CHUNK], F32, kind="ExternalInput")
    whh = nc.dram_tensor("whh", [H, 4 * H], F32, kind="ExternalInput")
    wih = nc.dram_tensor("wih", [128, H], F32, kind="ExternalInput")
    w0 = nc.dram_tensor("w0", [H, MID], F32, kind="ExternalInput")
    w1 = nc.dram_tensor("w1", [MID, H], F32, kind="ExternalInput")
    oww = nc.dram_tensor("oww", [H, NC_OUT], F32, kind="ExternalInput")
    bpack = nc.dram_tensor("bpack", [128, 4], F32, kind="ExternalInput")
    ident = nc.dram_tensor("ident", [128, 128], F32, kind="ExternalInput")
    outd = nc.dram_tensor("out", [OBS_LEN, NC_OUT, ROWS], F32,
                          kind="ExternalOutput")

    with tile.TileContext(nc) as tc:
        with tc.tile_pool(name="const", bufs=1) as const, \
             tc.tile_pool(name="state", bufs=1) as state, \
             tc.tile_pool(name="outs", bufs=2) as outs_p:

            whh_sb = const.tile([128, 4 * H], F32)
            nc.sync.dma_start(out=whh_sb[:], in_=whh[:])
            wih_sb = const.tile([128, H], F32)
            nc.sync.dma_start(out=wih_sb[:], in_=wih[:])
            w0_sb = const.tile([128, MID], F32)
            nc.sync.dma_start(out=w0_sb[:], in_=w0[:])
            w1_sb = const.tile([128, 2, H], F32)
            nc.sync.dma_start(out=w1_sb[:],
                              in_=w1.rearrange("(a p) h -> p a h", p=128))
            oww_sb = const.tile([128, NC_OUT], F32)
            nc.sync.dma_start(out=oww_sb[:], in_=oww[:])
            bp_sb = const.tile([128, 4], F32)
            nc.sync.dma_start(out=bp_sb[:], in_=bpack[:])
            id_sb = const.tile([128, 128], F32)
            nc.sync.dma_start(out=id_sb[:], in_=ident[:])
            # x-term moving operand, replicated into 4 partition groups
            xr_sb = const.tile([128, OBS_LEN, CHUNK], F32)
            for g in range(4):
                nc.sync.dma_start(out=xr_sb[32 * g:32 * g + 3, :, :],
                                  in_=xr[:])

            h_sb = state.tile([128, ROWS], F32)
            c_sb = state.tile([128, ROWS], BF16)

            for _rep in range(reps):
                # ---------- phase 1: transpose ph, mapping MLP ----------
                with tc.tile_pool(name="mlpsb", bufs=1) as mlpsb, \
                     tc.tile_pool(name="h1p", bufs=3) as h1p, \
                     tc.tile_pool(name="pst", bufs=2, space="PSUM") as pst, \
                     tc.tile_pool(name="ps1", bufs=2, space="PSUM") as ps1p, \
                     tc.tile_pool(name="ps0", bufs=2, space="PSUM") as ps0p:

                    ph_nat = mlpsb.tile([128, NTILE, H], F32, tag="ph_nat")
                    nc.sync.dma_start(
                        out=ph_nat[:],
                        in_=ph.rearrange("(n p) h -> p n h", p=128))
                    ph_t = mlpsb.tile([128, ROWS], F32, tag="ph_t")
                    for n in range(NTILE):
                        ptile = pst.tile([128, 128], F32)
                        nc.tensor.transpose(ptile[:], ph_nat[:, n, :],
                                            id_sb[:])
                        sl = ph_t[:, n * 128:(n + 1) * 128]
                        if n % 2 == 0:
                            nc.vector.tensor_copy(sl, ptile[:])
                        else:
                            nc.scalar.activation(sl, ptile[:], AF.Copy)

                    nc.vector.memset(c_sb[:], 0.0)

                    for j in range(NCH):
                        rs = slice(j * CHUNK, (j + 1) * CHUNK)
                        ps1 = ps1p.tile([128, 2, CHUNK], F32)
                        nc.tensor.matmul(ps1[:, 0, :], w0_sb[:, 0:128],
                                         ph_t[:, rs], start=True, stop=True)
                        nc.tensor.matmul(ps1[:, 1, :], w0_sb[:, 128:256],
                                         ph_t[:, rs], start=True, stop=True)
                        h1t = h1p.tile([128, 2, CHUNK], F32, tag="h1")
                        nc.scalar.activation(h1t[:, 0, :], ps1[:, 0, :],
                                             AF.Lrelu, bias=bp_sb[:, 0:1],
                                             alpha=0.01)
                        nc.scalar.activation(h1t[:, 1, :], ps1[:, 1, :],
                                             AF.Lrelu, bias=bp_sb[:, 1:2],
                                             alpha=0.01)
                        ps0 = ps0p.tile([128, CHUNK], F32)
                        nc.tensor.matmul(ps0[:], w1_sb[:, 0, :], h1t[:, 0, :],
                                         start=True, stop=False)
                        nc.tensor.matmul(ps0[:], w1_sb[:, 1, :], h1t[:, 1, :],
                                         start=False, stop=True)
                        nc.scalar.activation(h_sb[:, rs], ps0[:],
                                             AF.Identity, bias=bp_sb[:, 2:3])

                # ---------- phase 2: LSTM recurrence ----------
                with tc.tile_pool(name="psifo", bufs=2, space="PSUM") as psifo_p, \
                     tc.tile_pool(name="psg", bufs=1, space="PSUM") as psg_p, \
                     tc.tile_pool(name="pso", bufs=1, space="PSUM") as pso_p, \
                     tc.tile_pool(name="gsb", bufs=3) as gsb_p, \
                     tc.tile_pool(name="tmp", bufs=6) as tmp_p:

                    outstep = None
                    prev_outstep = None
                    for t in range(OBS_LEN):
                        prev_outstep = outstep
                        outstep = outs_p.tile([NC_OUT, ROWS], F32,
                                              tag="outstep")
                        for j in range(NCH):
                            rs = slice(j * CHUNK, (j + 1) * CHUNK)
                            # output projection of the PREVIOUS step reads
                            # h before this chunk's elementwise overwrites it
                            if t > 0:
                                pso = pso_p.tile([NC_OUT, CHUNK], F32)
                                nc.tensor.matmul(pso[:], oww_sb[:],
                                                 h_sb[:, rs],
                                                 start=True, stop=True)
                                nc.scalar.activation(
                                    prev_outstep[:, rs], pso[:], AF.Identity,
                                    bias=bp_sb[0:NC_OUT, 3:4])

                            psifo = psifo_p.tile([128, 3, CHUNK], F32)
                            psg = psg_p.tile([128, CHUNK], F32)
                            for gi in range(3):
                                nc.tensor.matmul(
                                    psifo[:, gi, :],
                                    whh_sb[:, gi * 128:(gi + 1) * 128],
                                    h_sb[:, rs], start=True, stop=False)
                            nc.tensor.matmul(psg[:], whh_sb[:, 384:512],
                                             h_sb[:, rs],
                                             start=True, stop=False)
                            xop = xr_sb[:, t, :]
                            for gi in range(3):
                                nc.tensor.matmul(
                                    psifo[:, gi, :],
                                    wih_sb[32 * gi:32 * gi + 3, :],
                                    xop[32 * gi:32 * gi + 3, :],
                                    start=False, stop=True,
                                    tile_position=(32 * gi, 0))
                            nc.tensor.matmul(psg[:], wih_sb[96:99, :],
                                             xop[96:99, :],
                                             start=False, stop=True,
                                             tile_position=(96, 0))

                            # evac i,f,o to SBUF (ACT), clamp there (DVE 4x)
                            ifo = gsb_p.tile([128, 3, CHUNK], BF16, tag="ifo")
                            nc.scalar.activation(ifo[:], psifo[:], AF.Copy)
                            nc.vector.tensor_scalar(
                                out=ifo[:], in0=ifo[:], scalar1=0.0,
                                scalar2=1.0, op0=OP.max, op1=OP.min)
                            # g: clamp straight out of PSUM
                            gt = gsb_p.tile([128, CHUNK], BF16, tag="g")
                            nc.vector.tensor_scalar(
                                out=gt[:], in0=psg[:], scalar1=1.0,
                                scalar2=-1.0, op0=OP.min, op1=OP.max)
                            # c' = f*c + i*g ; cc = clip(c') ; h = o*cc
                            t1 = tmp_p.tile([128, CHUNK], BF16, tag="t1")
                            nc.vector.tensor_tensor(
                                out=t1[:], in0=ifo[:, 1, :], in1=c_sb[:, rs],
                                op=OP.mult)
                            t2 = tmp_p.tile([128, CHUNK], BF16, tag="t2")
                            nc.vector.tensor_tensor(
                                out=t2[:], in0=ifo[:, 0, :], in1=gt[:],
                                op=OP.mult)
                            nc.vector.tensor_tensor(
                                out=c_sb[:, rs], in0=t1[:], in1=t2[:],
                                op=OP.add)
                            cc = tmp_p.tile([128, CHUNK], BF16, tag="cc")
                            nc.vector.tensor_scalar(
                                out=cc[:], in0=c_sb[:, rs], scalar1=1.0,
                                scalar2=-1.0, op0=OP.min, op1=OP.max)
                            nc.vector.tensor_tensor(
                                out=h_sb[:, rs], in0=ifo[:, 2, :], in1=cc[:],
                                op=OP.mult)
                        if t > 0:
                            nc.sync.dma_start(out=outd[t - 1], in_=prev_outstep[:])

                    # final step's output projection
                    for j in range(NCH):
                        rs = slice(j * CHUNK, (j + 1) * CHUNK)
                        pso = pso_p.tile([NC_OUT, CHUNK], F32)
                        nc.tensor.matmul(pso[:], oww_sb[:], h_sb[:, rs],
                                         start=True, stop=True)
                        nc.scalar.activation(outstep[:, rs], pso[:],
                                             AF.Identity,
                                             bias=bp_sb[0:NC_OUT, 3:4])
                    nc.sync.dma_start(out=outd[OBS_LEN - 1], in_=outstep[:])

    nc.finalize()
    return nc


def prep_inputs(obs_traj_rel, pred_lstm_hidden, map_w0, map_b0, map_w1,
                map_b1, w_ih, w_hh, b_ih, b_hh, out_w, out_b):
    """Host-side prep -> list of per-core input dicts."""
    f32 = np.float32
    bias = (np.asarray(b_ih, f32) + np.asarray(b_hh, f32))
    w_hh = np.asarray(w_hh, f32)
    w_ih = np.asarray(w_ih, f32)

    whh_stat = np.empty((H, 4 * H), f32)
    wih_stat = np.zeros((128, H), f32)
    for gi in range(4):
        sb = SRC_BLOCK[gi]
        s = (1.0 / 6.0) if gi < 3 else 1.0
        off = 0.5 if gi < 3 else 0.0
        whh_stat[:, gi * 128:(gi + 1) * 128] = w_hh[sb * 128:(sb + 1) * 128].T * s
        wih_stat[32 * gi + 0:32 * gi + 2, :] = \
            w_ih[sb * 128:(sb + 1) * 128, :].T * s
        wih_stat[32 * gi + 2, :] = bias[sb * 128:(sb + 1) * 128] * s + off

    bpack = np.zeros((128, 4), f32)
    bpack[:, 0] = np.asarray(map_b0, f32)[0:128]
    bpack[:, 1] = np.asarray(map_b0, f32)[128:256]
    bpack[:, 2] = np.asarray(map_b1, f32)
    bpack[0:NC_OUT, 3] = np.asarray(out_b, f32)

    obs = np.asarray(obs_traj_rel, f32)
    xs = np.concatenate([obs[0:1], obs[:-1]], axis=0)[:, :, 0:2]  # [T,B,2]
    ph_full = np.asarray(pred_lstm_hidden, f32)

    common = dict(
        whh=whh_stat, wih=wih_stat,
        w0=np.ascontiguousarray(np.asarray(map_w0, f32)),
        w1=np.ascontiguousarray(np.asarray(map_w1, f32)),
        oww=np.ascontiguousarray(np.asarray(out_w, f32)),
        bpack=bpack, ident=np.eye(128, dtype=f32),
    )
    in_maps = []
    for c in range(NCORES):
        bs = slice(c * BC, (c + 1) * BC)
        ph_core = np.ascontiguousarray(
            ph_full[:, bs, :].reshape(ROWS, H))
        x_core = xs[:, bs, :]                       # [T, BC, 2]
        xr_core = np.empty((3, OBS_LEN, CHUNK), f32)
        for t in range(OBS_LEN):
            for rep in range(CHUNK // BC):
                xr_core[0, t, rep * BC:(rep + 1) * BC] = x_core[t, :, 0]
                xr_core[1, t, rep * BC:(rep + 1) * BC] = x_core[t, :, 1]
        xr_core[2] = 1.0
        in_maps.append(dict(ph=ph_core, xr=xr_core, **common))
    return in_maps


def assemble_output(results):
    """Per-core [T, 2, ROWS] (k-major rows) -> full [T, K, B, 2]."""
    out = np.empty((OBS_LEN, K, B, NC_OUT), np.float32)
    for c, res in enumerate(results):
        o = res["out"].reshape(OBS_LEN, NC_OUT, K, BC)
        out[:, :, c * BC:(c + 1) * BC, :] = o.transpose(0, 2, 3, 1)
    return out


def kernel(**inputs):
    nc = build_nc(reps=1)
    in_maps = prep_inputs(**inputs)
    res = run_bass_kernel_spmd(nc, in_maps, core_ids=list(range(NCORES)))
    return assemble_output(res.results)


if __name__ == "__main__":
    import reference as R
    inputs = {k: np.asarray(v) for k, v in R.setup_inputs().items()}
    got = kernel(**inputs)
    import jax.numpy as jnp
    ref = np.asarray(R.reference(**{k: jnp.asarray(v) for k, v in inputs.items()}))
    err = np.abs(got - ref).max()
    rel = err / np.abs(ref).max()
    print(f"absmax={err:.4e} rel={rel:.4e}")
